# revision 15
# baseline (speedup 1.0000x reference)
"""Trainium2 Bass kernel for GraphTransformerLinkPredictor — v5.

v4 (4.14ms) trace: Pool serialized on 2480 DMA_INDIRECT x (1104ns Q7 +
309ns sequencer dispatch) = 11ns/row of Pool time. v5 switches every
data-dependent move to InstDMAGatherAnt (dma_gather):
  - ~8.4ns/descriptor Q7 loop but ONE dispatch per up-to-1024 rows;
  - queue_num q runs on Q7 core pair q -> with num_swdge_queues=4 and
    rotating queues, measured 1.83x overlap => ~4.6ns/row Pool time.
Costs taken to get there:
  - int16 indices are sign-extended (15-bit): gathers read from one
    25088-row quarter of the table per instruction. Edge subtiles are
    (target-tile x quarter)-pure (+~14% padded subtiles); per-tile
    processing batches issue <=4 quarter-run gathers each.
  - no dtype cast in dma_gather: kv/hf tables are bf16 (2x AllGather
    bytes, still chunk-overlapped).
  - pairs sorted into 16 (src-quarter x dst-quarter) buckets, padded to
    a shared per-bucket subtile count across cores.
Keeps from v4: group-major chunked AllGathers overlapping producers,
per-group q/sh slab pools for cross-layer overlap, per-group t_h/kvin
tensors, inline pass1/finish hooks.
"""

import math
import os
from contextlib import ExitStack

import numpy as np

P = 128
HID = 128
HEADS = 4
DH = 32
L = 2
EPS = 1e-5
NCORES = 8
NQUART = 4
GT = 7    # node tiles per linear-phase group
NQ = 4    # SWDGE queues
MAXSUB = 8  # max subtiles per gather instruction
QS = 4    # qsel matmuls per PSUM tile


def _groups(nt, g):
    out = []
    t0 = 0
    while t0 < nt:
        out.append((t0, min(g, nt - t0)))
        t0 += min(g, nt - t0)
    return out


def _pack_idx16(vals):
    """[n*128] row indices -> dma_gather idx layout [128, n*8] int16
    (slot i at [i%16, i//16], replicated across the 8 core groups)."""
    n = vals.shape[0]
    sc = n // 16
    lay = vals.reshape(sc, 16).T.astype(np.int16)   # [16, SC]
    return np.tile(lay, (8, 1))                     # [128, SC]


def _prep(inputs):
    import ml_dtypes

    bf = ml_dtypes.bfloat16

    x = np.ascontiguousarray(np.asarray(inputs["x"], dtype=np.float32))
    rw = np.ascontiguousarray(np.asarray(inputs["rw_diag"], dtype=np.float32))
    ei = np.asarray(inputs["edge_index"]).astype(np.int64)
    psrc = np.asarray(inputs["src"]).astype(np.int64)
    pdst = np.asarray(inputs["dst"]).astype(np.int64)

    N = x.shape[0]
    IN_C = x.shape[1]
    RWD = rw.shape[1]
    NT = math.ceil(N / (NCORES * P))
    NLOC = NT * P
    NPADT = NLOC * NCORES

    W_rwse = np.asarray(inputs["W_rwse"], np.float32)
    b_rwse = np.asarray(inputs["b_rwse"], np.float32)
    W_in = np.asarray(inputs["W_in"], np.float32)
    b_in = np.asarray(inputs["b_in"], np.float32)
    W1 = np.ascontiguousarray(W_in[:IN_C]).astype(bf)
    W2 = np.ascontiguousarray(W_rwse @ W_in[IN_C:]).astype(bf)
    bcat = (b_in + b_rwse @ W_in[IN_C:]).astype(np.float32)

    Wq = np.asarray(inputs["Wq"], np.float32)
    Wk = np.asarray(inputs["Wk"], np.float32)
    Wv = np.asarray(inputs["Wv"], np.float32)
    Ws = np.asarray(inputs["Ws"], np.float32)
    bq = np.asarray(inputs["bq"], np.float32)
    bk = np.asarray(inputs["bk"], np.float32)
    bv = np.asarray(inputs["bv"], np.float32)
    bs = np.asarray(inputs["bs"], np.float32)
    ln_g = np.asarray(inputs["ln_g"], np.float32)
    ln_b = np.asarray(inputs["ln_b"], np.float32)

    Wcat = [np.ascontiguousarray(np.concatenate(
        [Wq[l], Wk[l], Wv[l], Ws[l]], axis=1)).astype(bf) for l in range(L)]
    bqkvs = [np.concatenate([bq[l], bk[l], bv[l], bs[l]]) for l in range(L)]

    groups = _groups(NT, GT)
    NG = len(groups)
    # AllGather chunks = pairs of groups; global row layout is
    # CHUNK-major: [chunk][core][group-in-chunk][tile-in-group][128]
    CH = [tuple(range(i, min(i + 2, NG))) for i in range(0, NG, 2)]
    NCH = len(CH)
    ch_of_g = {g: ci for ci, ch in enumerate(CH) for g in ch}
    ch_lrows = [sum(groups[g][1] * P for g in ch) for ch in CH]
    ch_goff = {}
    for ci, ch in enumerate(CH):
        o = 0
        for g in ch:
            ch_goff[g] = o
            o += groups[g][1] * P
    chbase = np.zeros(NCH + 1, np.int64)
    for ci in range(NCH):
        chbase[ci + 1] = chbase[ci] + NCORES * ch_lrows[ci]
    # quarters = contiguous runs of whole chunks, each <= 32767 rows
    cq = [0]
    for ci in range(NCH):
        if chbase[ci + 1] - chbase[cq[-1]] > 32767:
            cq.append(ci)
    cq.append(NCH)
    NQe = len(cq) - 1
    assert NQe <= 8
    qbound = np.array([chbase[ci] for ci in cq], np.int64)
    qsizes = np.diff(qbound)
    assert qsizes.max() <= 32767, qsizes

    row = ei[0]
    col = ei[1]
    core_of = col // NLOC
    tile_of = (col % NLOC) // P

    # first pass: per-core per-tile counts to pick the slot permutation
    flat = core_of * NT + tile_of
    cnt = np.bincount(flat, minlength=NCORES * NT).reshape(NCORES, NT)
    perm = np.argsort(-cnt, axis=1, kind="stable")      # [NC, NT] slot->tile
    inv_perm = np.argsort(perm, axis=1)                 # [NC, NT] tile->slot

    # per-slot row base within a core's chunk segment
    srow_off = np.zeros(NT, np.int64)   # offset inside the core segment
    srow_ch = np.zeros(NT, np.int64)    # chunk of each slot
    for gi, (g0, gsz) in enumerate(groups):
        ci = ch_of_g[gi]
        for j in range(gsz):
            s = g0 + j
            srow_ch[s] = ci
            srow_off[s] = ch_goff[gi] + j * P
    ch_lrows_a = np.array(ch_lrows, np.int64)

    def slot_row(n):
        c = n // NLOC
        lo = n % NLOC
        s = inv_perm[c, lo // P]
        ci = srow_ch[s]
        return (chbase[ci] + c * ch_lrows_a[ci] + srow_off[s] + lo % P)

    # second pass: sort edges by (core, slot, src-quarter, src-row)
    msrow_all = slot_row(row)
    quart = np.searchsorted(qbound, msrow_all, side="right") - 1
    slot_of = np.take_along_axis(
        inv_perm[core_of], tile_of[None, :].T, axis=1).ravel() \
        if False else inv_perm[core_of, tile_of]
    order = np.lexsort((msrow_all, quart, slot_of, core_of))
    srow_m = msrow_all[order]
    scol = col[order]
    squart = quart[order]
    score_ = core_of[order]
    sslot = slot_of[order]

    # per (core, slot, quarter) counts -> shared subtile schedule
    cntq = np.zeros((NCORES, NT, NQe), np.int64)
    np.add.at(cntq, (score_, sslot, squart), 1)
    tcnt_q = np.ceil(cntq / P).astype(np.int64).max(axis=0)  # [NT, NQe]
    # ensure at least one subtile per slot overall (empty tiles)
    for s in range(NT):
        if tcnt_q[s].sum() == 0:
            tcnt_q[s, 0] = 1

    # processing order: superslots = pairs of slots; within a
    # superslot subtiles are QUARTER-major (so one gather instruction
    # covers both slots' run for a quarter), within a quarter
    # slot-major.
    NU = (NT + 1) // 2
    uslots = [tuple(t for t in (2 * u, 2 * u + 1) if t < NT)
              for u in range(NU)]
    sub_tile = []     # slot of each subtile
    sub_quart = []
    for u in range(NU):
        for q in range(NQe):
            for s in uslots[u]:
                for _ in range(int(tcnt_q[s, q])):
                    sub_tile.append(s)
                    sub_quart.append(q)
    ET = len(sub_tile)
    sub_tile = np.array(sub_tile)
    sub_quart = np.array(sub_quart)
    # subtile index ranges per superslot
    u_first = np.zeros(NU, np.int64)
    u_nsub = np.zeros(NU, np.int64)
    for u in range(NU):
        u_nsub[u] = sum(int(tcnt_q[s].sum()) for s in uslots[u])
    u_first[1:] = np.cumsum(u_nsub)[:-1]

    # gather instructions: per (superslot, quarter) run, split into
    # <=MAXSUB chunks. Each instr: (quarter, first_subtile, nsub)
    instrs = []
    st = 0
    for u in range(NU):
        for q in range(NQe):
            k = sum(int(tcnt_q[s, q]) for s in uslots[u])
            o = 0
            while o < k:
                take = min(MAXSUB, k - o)
                instrs.append((q, st + o, take))
                o += take
            st += k
    NI_E = len(instrs)

    # fill per-core edge slots + one-hots + int16 indices.
    # pad slots get index -1: the dma_gather ucode trims trailing -1s
    # per core, so padded rows are never fetched (stale SBUF data is
    # harmless: one-hot columns are zero there and scores can't
    # overflow bf16 exp).
    kvidx = np.zeros((NCORES, ET * P), np.int32)  # BISECT: 0-pad
    ohm = np.zeros((NCORES, ET * P, P), bf)
    ohtm = np.zeros((NCORES, ET, P, P), bf)  # [sub, node, edge]
    # per (core, slot, quarter) edge ranges in the sorted arrays
    gstart = np.zeros((NCORES, NT, NQe), np.int64)
    np.cumsum(cntq.ravel()[:-1], out=gstart.ravel()[1:])
    # subtile slot offsets in the new order
    sub_of_sq = {}
    st = 0
    for u in range(NU):
        for q in range(NQe):
            for s in uslots[u]:
                sub_of_sq[(s, q)] = st
                st += int(tcnt_q[s, q])
    for c in range(NCORES):
        for s in range(NT):
            for q in range(NQe):
                n = int(cntq[c, s, q])
                if n == 0:
                    continue
                e0 = int(gstart[c, s, q])
                o = sub_of_sq[(s, q)] * P
                sl = np.arange(o, o + n)
                kvidx[c, sl] = (srow_m[e0:e0 + n]
                                - qbound[q]).astype(np.int32)
                cl = (scol[e0:e0 + n] - c * NLOC).astype(np.int32)
                ohm[c, sl, cl % P] = 1
                ohtm[c, sl // P, cl % P, sl % P] = 1

    # int16 idx tensors per gather instruction, padded cols to MAXSUB*8
    SCW = MAXSUB * P // 16
    eidx16 = np.zeros((NCORES, NI_E, P, SCW), np.int16)
    for c in range(NCORES):
        for ii, (q, s0, nsub) in enumerate(instrs):
            vals = kvidx[c, s0 * P:(s0 + nsub) * P].copy()
            if vals[0] < 0:
                # all-pad instruction: keep one real row, the ucode
                # crashes on a fully-trimmed gather
                vals[0] = 0
            eidx16[c, ii, :, :nsub * 8] = _pack_idx16(vals)

    # oh: [sub, slot(part), node(free)] for the agg matmul lhsT;
    # oht: [sub, node(part), slot(free)] for the qsel matmul lhsT.
    oh_hw = np.ascontiguousarray(ohm.reshape(NCORES, ET, P, P))
    oht_hw = np.ascontiguousarray(ohtm)

    # node features (zero-padded, transposed tiles, slot order)
    xs = np.zeros((NCORES, NLOC, IN_C), np.float32)
    rws = np.zeros((NCORES, NLOC, RWD), np.float32)
    for c in range(NCORES):
        lo = c * NLOC
        hi = min(N, lo + NLOC)
        if hi > lo:
            xs[c, :hi - lo] = x[lo:hi]
            rws[c, :hi - lo] = rw[lo:hi]
    ci = np.arange(NCORES)[:, None]
    xT_hw = np.ascontiguousarray(
        xs.reshape(NCORES, NT, P, IN_C)[ci, perm].transpose(
            0, 1, 3, 2)).astype(bf)
    rwT_hw = np.ascontiguousarray(
        rws.reshape(NCORES, NT, P, RWD)[ci, perm].transpose(
            0, 1, 3, 2)).astype(bf)

    # ---- pairs: 16 (src-quarter, dst-quarter) buckets ----
    NPAIR = psrc.shape[0]
    PLOC = math.ceil(NPAIR / NCORES)
    plocs = [max(0, min(PLOC, NPAIR - c * PLOC)) for c in range(NCORES)]
    mpsrc = slot_row(psrc)
    mpdst = slot_row(pdst)
    qsrc = np.searchsorted(qbound, mpsrc, side="right") - 1
    qdst = np.searchsorted(qbound, mpdst, side="right") - 1
    bidx = qsrc * NQe + qdst                        # [NPAIR] buckets
    NBK = NQe * NQe
    # per-core bucket membership and shared per-bucket subtile counts
    bmax = np.zeros(NBK, np.int64)
    core_b = []
    for c in range(NCORES):
        lo, hi = c * PLOC, c * PLOC + plocs[c]
        bc = bidx[lo:hi]
        cnts = np.bincount(bc, minlength=NBK)
        bmax = np.maximum(bmax, np.ceil(cnts / P).astype(np.int64))
        core_b.append((lo, bc))
    bsub = bmax                      # shared subtiles per bucket
    NPS = int(bsub.sum())
    # pair gather instructions per bucket (src side + dst side share
    # slots): per bucket, ceil(nsub/MAXSUB) instrs per side
    pinstrs = []                     # (qsrc, qdst, first_subtile, nsub)
    bfirst = np.zeros(NBK, np.int64)
    st = 0
    for b in range(NBK):
        bfirst[b] = st
        o = 0
        while o < int(bsub[b]):
            take = min(MAXSUB, int(bsub[b]) - o)
            pinstrs.append((b // NQe, b % NQe, st + o, take))
            o += take
        st += int(bsub[b])
    NI_P = len(pinstrs)

    psidx = np.zeros((NCORES, NPS * P), np.int32)
    pdidx = np.zeros((NCORES, NPS * P), np.int32)
    pperm = np.full((NCORES, NPS * P), -1, np.int64)  # slot -> local pair
    for c in range(NCORES):
        lo, bc = core_b[c]
        for b in range(NBK):
            sel = np.where(bc == b)[0]          # local pair indices
            o = int(bfirst[b]) * P
            psidx[c, o:o + len(sel)] = (mpsrc[lo + sel]
                                        - qbound[b // NQe])
            pdidx[c, o:o + len(sel)] = (mpdst[lo + sel]
                                        - qbound[b % NQe])
            pperm[c, o:o + len(sel)] = sel
    pidx16 = np.zeros((NCORES, NI_P, P, 2 * SCW), np.int16)
    for c in range(NCORES):
        for ii, (qs, qd, s0, nsub) in enumerate(pinstrs):
            sv = psidx[c, s0 * P:(s0 + nsub) * P].copy()
            dv = pdidx[c, s0 * P:(s0 + nsub) * P].copy()
            if sv[0] < 0:
                sv[0] = 0
            if dv[0] < 0:
                dv[0] = 0
            pidx16[c, ii, :, :nsub * 8] = _pack_idx16(sv)
            pidx16[c, ii, :, SCW:SCW + nsub * 8] = _pack_idx16(dv)

    return dict(
        N=N, IN_C=IN_C, RWD=RWD, NT=NT, NLOC=NLOC, NPADT=NPADT,
        NQe=NQe, qbound=qbound, qsizes=qsizes, NBK=NBK,
        CH=CH, ch_of_g=ch_of_g, ch_lrows=ch_lrows, ch_goff=ch_goff,
        chbase=chbase,
        ET=ET, NI_E=NI_E, instrs=instrs, SCW=SCW,
        NG=NG, groups=groups, NU=NU, uslots=uslots,
        u_first=u_first, u_nsub=u_nsub,

        sub_tile=sub_tile, sub_quart=sub_quart, tcnt_q=tcnt_q,
        NPS=NPS, NI_P=NI_P, pinstrs=pinstrs, bsub=bsub, bfirst=bfirst,
        pperm=pperm, PLOC=PLOC, plocs=plocs, NPAIR=NPAIR,
        W1=W1, W2=W2, bcat=bcat, Wcat=Wcat, bqkvs=bqkvs,
        ln_g=ln_g, ln_b=ln_b,
        xT=xT_hw, rwT=rwT_hw, eidx=eidx16, oh=oh_hw, oht=oht_hw,
        pidx=pidx16,
    )
